# revision 3
# baseline (speedup 1.0000x reference)
"""TP=8 LSTM decoder v3: v1 flat/ncfw structure + bf16 gate matmuls +
mel-sliced y (each core computes y[:, s*64:(s+1)*64] via per-core weight
slices; SPMD instruction streams stay identical)."""

import numpy as np

B = 64
H = 2048
MEL = 512
NC = 8
HS = H // NC
GS = 4 * HS
KCH = H // 128
YS = MEL // NC


def build_nc_v3(n_steps: int):
    import concourse.bass as bass
    import concourse.bacc as bacc
    import concourse.mybir as mybir
    from concourse.bass import ts

    f32 = mybir.dt.float32
    bf16 = mybir.dt.bfloat16
    T = n_steps

    nc = bacc.Bacc("TRN2", target_bir_lowering=False, debug=False,
                   num_devices=NC)

    wct_d = nc.dram_tensor("wct", [128, KCH * GS], bf16, kind="ExternalInput")
    woy_d = nc.dram_tensor("woy", [128, KCH * YS], bf16, kind="ExternalInput")
    h0t_d = nc.dram_tensor("h0t", [128, KCH * B], bf16, kind="ExternalInput")
    c0s_d = nc.dram_tensor("c0s", [B, HS], f32, kind="ExternalInput")
    bps_d = nc.dram_tensor("bps", [1, GS], bf16, kind="ExternalInput")
    boy_d = nc.dram_tensor("boy", [1, YS], bf16, kind="ExternalInput")
    ones_d = nc.dram_tensor("ones", [1, B], bf16, kind="ExternalInput")
    ident_d = nc.dram_tensor("ident", [B, B], bf16, kind="ExternalInput")
    yout_d = nc.dram_tensor("yout", [512, B, YS], f32, kind="ExternalOutput")

    cc_in = nc.dram_tensor("cc_in", [128, 128], bf16)
    cc_out = [nc.dram_tensor(f"cc_out{b}", [NC * 128, 128], bf16,
                             addr_space="Shared") for b in range(2)]

    ctx_list = []

    def sb(name, shape, dt):
        t = nc.sbuf_tensor(name, shape, dt)
        ctx_list.append(t)
        return t.__enter__()

    def ps(name, shape, dt):
        t = nc.psum_tensor(name, shape, dt)
        ctx_list.append(t)
        return t.__enter__()

    def sem(name):
        t = nc.semaphore(name)
        ctx_list.append(t)
        return t.__enter__()

    s_wct = sb("s_wct", [128, KCH * GS], bf16)
    s_woy = sb("s_woy", [128, KCH * YS], bf16)
    s_hT = [sb("s_hT0", [128, KCH * B], bf16), sb("s_hT1", [128, KCH * B], bf16)]
    s_c = [sb("s_c0", [B, HS], f32), sb("s_c1", [B, HS], f32)]
    s_sif = sb("s_sif", [B, 2 * HS], f32)
    s_tg = sb("s_tg", [B, HS], f32)
    s_so = sb("s_so", [B, HS], f32)
    s_tc = sb("s_tc", [B, HS], f32)
    s_t1 = sb("s_t1", [B, HS], f32)
    s_t2 = sb("s_t2", [B, HS], f32)
    s_h = sb("s_h", [B, HS], bf16)
    s_stage = sb("s_stage", [128, 128], bf16)
    s_y = [sb("s_y0", [B, YS], f32), sb("s_y1", [B, YS], f32)]
    s_bps = sb("s_bps", [1, GS], bf16)
    s_boy = sb("s_boy", [1, YS], bf16)
    s_ones = sb("s_ones", [1, B], bf16)
    s_ident = sb("s_ident", [B, B], bf16)

    p_g0 = ps("p_g0", [B, 512], f32)    # gates j 0:512  (i|f)
    p_g1 = ps("p_g1", [B, 512], f32)    # gates j 512:1024 (g|o)
    p_y = ps("p_y", [B, YS], f32)
    p_tr = ps("p_tr", [128, 128], bf16)

    s_pre = sem("s_pre")
    s_gates = sem("s_gates")
    s_act1 = sem("s_act1")
    s_dvec = sem("s_dvec")
    s_act2 = sem("s_act2")
    s_dveh = sem("s_dveh")
    s_tr = sem("s_tr")
    s_stg = sem("s_stg")
    s_yv = sem("s_yv")
    s_ycp = sem("s_ycp")
    s_ydma = sem("s_ydma")
    s_ccin = sem("s_ccin")
    s_cc = sem("s_cc")
    s_hin = sem("s_hin")

    N_PRE = 8

    with nc.Block() as block:

        @block.sync
        def _(sync):
            sync.dma_start(out=s_wct[:, :], in_=wct_d[:, :]).then_inc(s_pre, 16)
            sync.dma_start(out=s_hT[0][:, :], in_=h0t_d[:, :]).then_inc(s_pre, 16)
            sync.dma_start(out=s_c[0][:, :], in_=c0s_d[:, :]).then_inc(s_pre, 16)
            sync.dma_start(out=s_bps[:, :], in_=bps_d[:, :]).then_inc(s_pre, 16)
            sync.dma_start(out=s_boy[:, :], in_=boy_d[:, :]).then_inc(s_pre, 16)
            sync.dma_start(out=s_ones[:, :], in_=ones_d[:, :]).then_inc(s_pre, 16)
            sync.dma_start(out=s_ident[:, :], in_=ident_d[:, :]).then_inc(s_pre, 16)
            sync.dma_start(out=s_woy[:, :], in_=woy_d[:, :]).then_inc(s_pre, 16)
            for t in range(1, T + 1):
                if t <= T - 1:
                    # scatter cc_out blocks into hT buffer columns
                    sync.wait_ge(s_cc, t)
                    if t >= 2:
                        sync.wait_ge(s_hin, 16 * (t - 1))
                    sync.dma_start(
                        out=s_hT[t % 2][:, :].rearrange("p (r c) -> p r c", r=NC),
                        in_=cc_out[t % 2][:, :].rearrange("(r p) c -> p r c", r=NC),
                    ).then_inc(s_hin, 16)
                sync.wait_ge(s_ycp, t)
                if t >= 2:
                    sync.wait_ge(s_ydma, 16 * (t - 1))
                sync.dma_start(
                    out=yout_d[t - 1, :, :],
                    in_=s_y[(t - 1) % 2][:, :]
                ).then_inc(s_ydma, 16)

        @block.tensor
        def _(pe):
            pe.wait_ge(s_pre, 16 * N_PRE)
            for t in range(1, T + 1):
                rb = (t - 1) % 2
                hbuf = s_hT[rb]
                if t >= 2:
                    pe.wait_ge(s_hin, 16 * (t - 1))
                    pe.wait_ge(s_stg, t - 1)   # p_tr WAR
                    pe.wait_ge(s_ycp, t - 1)   # p_y WAR
                if t <= T - 1:
                    nc.tensor.matmul(p_g0[:, :], s_ones[:, :],
                                     s_bps[:, 0:512], start=True, stop=False)
                    nc.tensor.matmul(p_g1[:, :], s_ones[:, :],
                                     s_bps[:, 512:1024], start=True, stop=False)
                    for k in range(KCH):
                        lhsT = hbuf[:, ts(k, B)]
                        last = k == KCH - 1
                        nc.tensor.matmul(p_g0[:, :], lhsT,
                                         s_wct[:, k * GS: k * GS + 512],
                                         start=False, stop=last)
                        mm = nc.tensor.matmul(p_g1[:, :], lhsT,
                                              s_wct[:, k * GS + 512: (k + 1) * GS],
                                              start=False, stop=last)
                    mm.then_inc(s_gates, 1)
                # y(t-1) mel slice
                nc.tensor.matmul(p_y[:, :], s_ones[:, :], s_boy[:, :],
                                 start=True, stop=False)
                for k in range(KCH):
                    mm = nc.tensor.matmul(p_y[:, :], hbuf[:, ts(k, B)],
                                          s_woy[:, ts(k, YS)],
                                          start=False, stop=(k == KCH - 1))
                mm.then_inc(s_yv, 1)
                if t <= T - 1:
                    pe.wait_ge(s_dveh, t)
                    nc.tensor.transpose(p_tr[:, 0:B], s_h[:, 0:128],
                                        s_ident[:, :])
                    nc.tensor.transpose(p_tr[:, B:128], s_h[:, 128:256],
                                        s_ident[:, :]).then_inc(s_tr, 1)

        @block.scalar
        def _(act):
            act.wait_ge(s_pre, 16 * N_PRE)
            Sig = mybir.ActivationFunctionType.Sigmoid
            Tanh = mybir.ActivationFunctionType.Tanh
            for t in range(1, T):
                act.wait_ge(s_gates, t)
                nc.scalar.activation(s_sif[:, :], p_g0[:, :], Sig)
                nc.scalar.activation(s_tg[:, :], p_g1[:, 0:HS], Tanh)\
                    .then_inc(s_act1, 1)
                nc.scalar.activation(s_so[:, :], p_g1[:, HS:2 * HS], Sig)
                act.wait_ge(s_dvec, t)
                nc.scalar.activation(s_tc[:, :], s_c[t % 2][:, :], Tanh)\
                    .then_inc(s_act2, 1)
                act.wait_ge(s_stg, t)
                if t >= 2:
                    act.wait_ge(s_ccin, 16 * (t - 1))
                act.dma_start(out=cc_in[:, :], in_=s_stage[:, :])\
                    .then_inc(s_ccin, 16)

        @block.vector
        def _(dve):
            dve.wait_ge(s_pre, 16 * N_PRE)
            mult = mybir.AluOpType.mult
            add = mybir.AluOpType.add
            for t in range(1, T + 1):
                if t <= T - 1:
                    dve.wait_ge(s_act1, t)
                    nc.vector.scalar_tensor_tensor(
                        s_t1[:, :], s_sif[:, 0:HS], 1.0, s_tg[:, :], mult, mult)
                    nc.vector.scalar_tensor_tensor(
                        s_t2[:, :], s_sif[:, HS:2 * HS], 1.0,
                        s_c[(t - 1) % 2][:, :], mult, mult)
                    dve.drain()
                    nc.vector.scalar_tensor_tensor(
                        s_c[t % 2][:, :], s_t1[:, :], 1.0, s_t2[:, :],
                        mult, add).then_inc(s_dvec, 1)
                    dve.drain()
                    dve.wait_ge(s_act2, t)
                    nc.vector.scalar_tensor_tensor(
                        s_h[:, :], s_so[:, :], 1.0, s_tc[:, :], mult, mult)\
                        .then_inc(s_dveh, 1)
                dve.wait_ge(s_yv, t)
                if t >= 3:
                    dve.wait_ge(s_ydma, 16 * (t - 2))
                nc.vector.tensor_copy(s_y[(t - 1) % 2][:, :], p_y[:, :])\
                    .then_inc(s_ycp, 1)
                if t <= T - 1:
                    dve.wait_ge(s_tr, t)
                    if t >= 2:
                        dve.wait_ge(s_ccin, 16 * (t - 1))
                    nc.vector.tensor_copy(s_stage[:, :], p_tr[:, :])\
                        .then_inc(s_stg, 1)

        @block.gpsimd
        def _(gpsimd):
            gpsimd.wait_ge(s_pre, 16 * N_PRE)
            for t in range(1, T):
                gpsimd.wait_ge(s_ccin, 16 * t)
                import concourse.mybir as mybir2
                gpsimd.collective_compute(
                    "AllGather",
                    mybir2.AluOpType.bypass,
                    replica_groups=[list(range(NC))],
                    ins=[cc_in.ap().opt()],
                    outs=[cc_out[t % 2].ap().opt()],
                ).then_inc(s_cc)

    for c in reversed(ctx_list):
        c.__exit__(None, None, None)

    nc.compile()
    return nc


def _sigmoid(x):
    return 1.0 / (1.0 + np.exp(-x))


def _bf16(x):
    import ml_dtypes
    return np.ascontiguousarray(np.asarray(x, np.float32).astype(ml_dtypes.bfloat16))


def prepare_inputs_v3(inputs: dict):
    h0 = np.asarray(inputs["h0"])[0].astype(np.float32)
    c0 = np.asarray(inputs["c0"])[0].astype(np.float32)
    W_ih = np.asarray(inputs["W_ih"]).astype(np.float32)
    W_hh = np.asarray(inputs["W_hh"]).astype(np.float32)
    b = (np.asarray(inputs["b_ih"]) + np.asarray(inputs["b_hh"])).astype(np.float32)
    W_out = np.asarray(inputs["W_out"]).astype(np.float32)
    b_out = np.asarray(inputs["b_out"]).astype(np.float32)

    W_comb = W_hh + W_ih @ W_out
    bp = b + W_ih @ b_out

    gates0 = h0 @ W_hh.T + b
    i0, f0, g0, o0 = np.split(gates0, 4, axis=1)
    c1 = _sigmoid(f0) * c0 + _sigmoid(i0) * np.tanh(g0)
    h1 = _sigmoid(o0) * np.tanh(c1)

    hT = np.ascontiguousarray(h1.T)
    h0t = hT.reshape(KCH, 128, B).transpose(1, 0, 2).reshape(128, KCH * B)
    WoutT = np.ascontiguousarray(W_out.T)

    in_maps = []
    for s in range(NC):
        rows = np.concatenate(
            [np.arange(g * H + s * HS, g * H + (s + 1) * HS)
             for g in range(4)])
        WcT = np.ascontiguousarray(W_comb[rows, :].T)
        wct = WcT.reshape(KCH, 128, GS).transpose(1, 0, 2).reshape(128, KCH * GS)
        woyT = np.ascontiguousarray(WoutT[:, s * YS:(s + 1) * YS])
        woy = woyT.reshape(KCH, 128, YS).transpose(1, 0, 2).reshape(128, KCH * YS)
        in_maps.append({
            "wct": _bf16(wct),
            "woy": _bf16(woy),
            "h0t": _bf16(h0t),
            "c0s": np.ascontiguousarray(c1[:, s * HS:(s + 1) * HS]),
            "bps": _bf16(bp[rows][None, :]),
            "boy": _bf16(b_out[None, s * YS:(s + 1) * YS]),
            "ones": _bf16(np.ones((1, B), np.float32)),
            "ident": _bf16(np.eye(B, dtype=np.float32)),
        })
    return in_maps


TRACE = False
LAST_EXEC_NS = None


def kernel(**inputs):
    global LAST_EXEC_NS
    T = 512
    nc = build_nc_v3(T)
    in_maps = prepare_inputs_v3(inputs)
    from concourse import bass_utils
    res = bass_utils.run_bass_kernel_spmd(nc, in_maps, core_ids=list(range(NC)),
                                          trace=TRACE)
    if TRACE:
        LAST_EXEC_NS = res.exec_time_ns
    y = np.empty((B, T, MEL), np.float32)
    for s in range(NC):
        y[:, :, s * YS:(s + 1) * YS] = np.transpose(
            res.results[s]["yout"][:T], (1, 0, 2))
    return y
